# revision 19
# baseline (speedup 1.0000x reference)
"""Deformable 3x3 conv (DCNv1) on 8 TRN2 NeuronCores — raw Bass implementation.

Sharding: data-parallel over (image n, spatial half) -> 8 shards, no
collectives.  Each core:
  1. builds a bf16 4-corner gather table [S+65, 4C] in DRAM from its image
     (four casting DRAM->DRAM DMAs; entry e holds xT rows e-65, e-64, e-1, e
     = the corners (y0,x0),(y0,x1),(y1,x0),(y1,x1) for s00 = e-65),
  2. computes bilinear indices/weights on DVE (floor via +bias & int
     truncation with rounding fixup; out-of-bounds validity folded into the
     corner weights; indices clipped into the table),
  3. per 128-position tile: 9 indirect DMAs (one per kernel position)
     gather the 4C corner row per partition,
  4. corner weights applied as per-partition scalars (DVE: slots 0-19,
     ACT: slots 20-35), two slot-sum adds on DVE (software-pipelined,
     one pipeline drain per tile),
  5. one DMA-transpose per tile (sync engine) writes val^T into the matmul
     rhs buffer; PE accumulates out = W^T @ val over the 2304-deep
     contraction in PSUM (bf16).
Host reassembles the [N, O, H, W] output from per-core [O, 2048] shards.
"""

import os
import sys

import numpy as np

for _p in ("/opt/trn_rl_repo", "/root/.axon_site/_ro/trn_rl_repo"):
    if os.path.isdir(_p) and _p not in sys.path:
        sys.path.insert(0, _p)

import concourse.bass as bass
import concourse.mybir as mybir
from concourse.bass_utils import run_bass_kernel_spmd

AL = mybir.AluOpType
F32 = mybir.dt.float32
BF16 = mybir.dt.bfloat16
I32 = mybir.dt.int32

# problem dims
N, C, H, W, O = 4, 256, 64, 64, 256
S = H * W            # 4096 pixels per image
K = 9                # 3x3 kernel positions
P_SH = S // 2        # 2048 output positions per core
NPT = P_SH // 128    # 16 position-tiles
ST = 4               # ptiles per supertile (matmul rhs free = 512)
NST = NPT // ST      # 4
NJ = (K * C) // 128  # 18 contraction sub-tiles
M = NPT * K          # 144

TROWS = S + 65       # table rows; entry e covers s00 = e-65 (e >= 0 for any
                     # index with at least one valid corner: min s00 = -65)
ROW = 4 * C          # elements per table row
IDX_LO = 0.0
IDX_HI = float(S + 64)
BIAS = 64.0
S_CONST = -(BIAS * 64.0 + BIAS) + 65.0  # -4095

NDVE_SCL = 20        # corner-weight scalings on DVE; rest (16) on ACT
_GRAPH_CACHE = {}


def _emit(nc):
    xt = nc.dram_tensor("xt", [S, C], F32, kind="ExternalInput").ap()
    wt = nc.dram_tensor("wt", [K * C, O], F32, kind="ExternalInput").ap()
    offs = nc.dram_tensor("offs", [P_SH, 18], F32, kind="ExternalInput").ap()
    basey = nc.dram_tensor("basey", [P_SH, K], F32, kind="ExternalInput").ap()
    basex = nc.dram_tensor("basex", [P_SH, K], F32, kind="ExternalInput").ap()
    out = nc.dram_tensor("out", [O, P_SH], F32, kind="ExternalOutput").ap()
    table = nc.dram_tensor("table", [TROWS, ROW], BF16).ap()

    def sb(name, shape, dtype):
        return nc.alloc_sbuf_tensor(name, list(shape), dtype).ap()

    wts = sb("wts", [128, NJ * O], BF16)
    offt = sb("offt", [128, NPT * 18], F32)
    byt = sb("byt", [128, M], F32)
    bxt = sb("bxt", [128, M], F32)
    ztile = sb("ztile", [128, C], BF16)
    tPY = sb("tPY", [128, M], F32)
    tPX = sb("tPX", [128, M], F32)
    tF = sb("tF", [128, M], F32)
    tFR = sb("tFR", [128, M], F32)
    tFX = sb("tFX", [128, M], F32)
    tFRX = sb("tFRX", [128, M], F32)
    tA = sb("tA", [128, M], F32)
    tB = sb("tB", [128, M], F32)
    tC = sb("tC", [128, M], F32)
    tD = sb("tD", [128, M], F32)
    tI = sb("tI", [128, M], I32)
    w36 = sb("w36", [128, NPT * 36], F32)
    idx = sb("idx", [128, NPT * K], I32)
    g2 = [sb(f"g{b}", [128, K * ROW], BF16) for b in range(2)]       # 9216
    va2 = [sb(f"va{b}", [128, K * 2 * C], BF16) for b in range(2)]   # 4608
    val2 = [sb(f"val{b}", [128, K * C], BF16) for b in range(2)]     # 2304
    valt2 = [sb(f"valt{b}", [128, NJ * 512], BF16) for b in range(2)]
    ob2 = [sb(f"ob{b}", [128, 512], F32) for b in range(2)]

    po2 = [nc.alloc_psum_tensor(f"po{i}", [128, 512], F32).ap() for i in range(2)]

    with (
        nc.Block() as block,
        nc.semaphore("sZ") as sZ,
        nc.semaphore("sSet") as sSet,
        nc.semaphore("sTabG") as sTabG,
        nc.semaphore("sTabZ") as sTabZ,
        nc.semaphore("sIdx") as sIdx,
        nc.semaphore("sGatA") as sGatA,
        nc.semaphore("sGatB") as sGatB,
        nc.semaphore("sBl") as sBl,
        nc.semaphore("sTpD0") as sTpD0,
        nc.semaphore("sTpD1") as sTpD1,
        nc.semaphore("sScl") as sScl,
        nc.semaphore("sMM") as sMM,
        nc.semaphore("sOB") as sOB,
        nc.semaphore("sOD") as sOD,
    ):
        sTpD = [sTpD0, sTpD1]

        @block.gpsimd
        def _(ge):
            ge.dma_start(
                out=wts[:].rearrange("p (j o) -> p j o", j=NJ),
                in_=wt.rearrange("(j p) o -> p j o", p=128),
            ).then_inc(sSet, 16)
            ge.dma_start(
                out=offt[:].rearrange("p (t j) -> p t j", t=NPT),
                in_=offs.rearrange("(t p) j -> p t j", p=128),
            ).then_inc(sSet, 16)
            ge.dma_start(
                out=byt[:].rearrange("p (t k) -> p t k", t=NPT),
                in_=basey.rearrange("(t p) k -> p t k", p=128),
            ).then_inc(sSet, 16)
            ge.dma_start(
                out=bxt[:].rearrange("p (t k) -> p t k", t=NPT),
                in_=basex.rearrange("(t p) k -> p t k", p=128),
            ).then_inc(sSet, 16)
            # table build (f32 -> bf16 cast, DRAM->DRAM):
            # entry e: cols0 = xT[e-65], cols1 = xT[e-64], cols2 = xT[e-1], cols3 = xT[e]
            ge.dma_start(out=table[65 : S + 65, 0:C], in_=xt[:, :]).then_inc(sTabG, 16)
            ge.dma_start(out=table[64 : S + 64, C : 2 * C], in_=xt[:, :]).then_inc(
                sTabG, 16
            )
            ge.dma_start(out=table[1 : S + 1, 2 * C : 3 * C], in_=xt[:, :]).then_inc(
                sTabG, 16
            )
            ge.dma_start(out=table[0:S, 3 * C : 4 * C], in_=xt[:, :]).then_inc(
                sTabG, 16
            )

            for t in range(NPT):
                b = t % 2
                if t == 0:
                    ge.wait_ge(sTabG, 64)   # 4 table build DMAs
                    ge.wait_ge(sTabZ, 112)  # 7 zero edge writes
                    ge.wait_ge(sIdx, 1)
                if t >= 2:
                    ge.wait_ge(sBl, t - 1)  # g2[b] consumed by blend t-2
                for k in range(K):
                    ge.indirect_dma_start(
                        out=g2[b][:, k * ROW : (k + 1) * ROW],
                        out_offset=None,
                        in_=table[:, :],
                        in_offset=bass.IndirectOffsetOnAxis(
                            ap=idx[:, t * K + k : t * K + k + 1], axis=0
                        ),
                    ).then_inc(sGatA if b == 0 else sGatB, 16)

        @block.sync
        def _(sy):
            sy.wait_ge(sZ, 1)
            # zero the slots whose source rows fall outside the image
            sy.dma_start(out=table[0:65, 0:C], in_=ztile[0:65, :]).then_inc(sTabZ, 16)
            sy.dma_start(out=table[0:64, C : 2 * C], in_=ztile[0:64, :]).then_inc(
                sTabZ, 16
            )
            sy.dma_start(
                out=table[S + 64 : S + 65, C : 2 * C], in_=ztile[0:1, :]
            ).then_inc(sTabZ, 16)
            sy.dma_start(out=table[0:1, 2 * C : 3 * C], in_=ztile[0:1, :]).then_inc(
                sTabZ, 16
            )
            sy.dma_start(
                out=table[S + 1 : S + 65, 2 * C : 3 * C], in_=ztile[0:64, :]
            ).then_inc(sTabZ, 16)
            sy.dma_start(
                out=table[S : S + 33, 3 * C : 4 * C], in_=ztile[0:33, :]
            ).then_inc(sTabZ, 16)
            sy.dma_start(
                out=table[S + 33 : S + 65, 3 * C : 4 * C], in_=ztile[0:32, :]
            ).then_inc(sTabZ, 16)
            for t in range(NPT):
                stb = (t // ST) % 2
                q = t % ST
                if q == 0 and t // ST >= 2:
                    # valt2[stb] consumed by matmuls of supertile t//ST - 2
                    sy.wait_ge(sMM, 2 * (t // ST - 1))
                sy.wait_ge(sBl, t + 1)
                sy.dma_start_transpose(
                    out=valt2[stb][:]
                    .rearrange("p (j w) -> p j w", w=512)[
                        :, :, q * 128 : (q + 1) * 128
                    ],
                    in_=val2[t % 2][:, :],
                ).then_inc(sTpD0 if t % 2 == 0 else sTpD1, 16)
                if q == ST - 1:
                    st = t // ST
                    for oh in range(2):
                        sy.wait_ge(sOB, 2 * st + oh + 1)
                        sy.dma_start(
                            out=out[
                                oh * 128 : (oh + 1) * 128, st * 512 : (st + 1) * 512
                            ],
                            in_=ob2[oh][:, :],
                        ).then_inc(sOD, 16)
            sy.wait_ge(sOD, 16 * 2 * NST)

        @block.vector
        def _(v):
            v.memset(ztile[:, :], 0.0).then_inc(sZ, 1)
            v.wait_ge(sSet, 64)

            off3 = offt[:].rearrange("p (tk two) -> p tk two", two=2)
            dy = off3[:, :, 0:1]
            dx = off3[:, :, 1:2]

            def u3(t_):
                return t_[:].unsqueeze(2)

            def tt(**kw):
                v.drain()
                return v.tensor_tensor(**kw)

            def ts(**kw):
                v.drain()
                return v.tensor_scalar(**kw)

            def cp(**kw):
                v.drain()
                return v.tensor_copy(**kw)

            # py/px with bias
            tt(out=u3(tPY), in0=dy, in1=u3(byt), op=AL.add)
            tt(out=u3(tPX), in0=dx, in1=u3(bxt), op=AL.add)
            # floor(py) via int truncation + fixup
            cp(out=tI[:], in_=tPY[:])
            cp(out=tF[:], in_=tI[:])
            tt(out=tFR[:], in0=tPY[:], in1=tF[:], op=AL.subtract)
            ts(out=tA[:], in0=tFR[:], scalar1=0.0, scalar2=None, op0=AL.is_lt)
            tt(out=tF[:], in0=tF[:], in1=tA[:], op=AL.subtract)
            tt(out=tFR[:], in0=tFR[:], in1=tA[:], op=AL.add)
            # floor(px)
            cp(out=tI[:], in_=tPX[:])
            cp(out=tFX[:], in_=tI[:])
            tt(out=tFRX[:], in0=tPX[:], in1=tFX[:], op=AL.subtract)
            ts(out=tA[:], in0=tFRX[:], scalar1=0.0, scalar2=None, op0=AL.is_lt)
            tt(out=tFX[:], in0=tFX[:], in1=tA[:], op=AL.subtract)
            tt(out=tFRX[:], in0=tFRX[:], in1=tA[:], op=AL.add)
            # validity masks (biased domain: valid y0 in [64, 127])
            ts(out=tA[:], in0=tF[:], scalar1=64.0, scalar2=None, op0=AL.is_ge)
            ts(out=tB[:], in0=tF[:], scalar1=127.0, scalar2=None, op0=AL.is_le)
            tt(out=tA[:], in0=tA[:], in1=tB[:], op=AL.mult)  # vy0
            ts(out=tB[:], in0=tF[:], scalar1=63.0, scalar2=None, op0=AL.is_ge)
            ts(out=tC[:], in0=tF[:], scalar1=126.0, scalar2=None, op0=AL.is_le)
            tt(out=tB[:], in0=tB[:], in1=tC[:], op=AL.mult)  # vy1
            # ay0 = (1-wy)*vy0 -> tC ; ay1 = wy*vy1 -> tD
            ts(out=tC[:], in0=tFR[:], scalar1=-1.0, scalar2=1.0, op0=AL.mult, op1=AL.add)
            tt(out=tC[:], in0=tC[:], in1=tA[:], op=AL.mult)
            tt(out=tD[:], in0=tFR[:], in1=tB[:], op=AL.mult)
            # vx0 -> tA ; vx1 -> tB (tPY reused as scratch)
            ts(out=tA[:], in0=tFX[:], scalar1=64.0, scalar2=None, op0=AL.is_ge)
            ts(out=tB[:], in0=tFX[:], scalar1=127.0, scalar2=None, op0=AL.is_le)
            tt(out=tA[:], in0=tA[:], in1=tB[:], op=AL.mult)
            ts(out=tB[:], in0=tFX[:], scalar1=63.0, scalar2=None, op0=AL.is_ge)
            ts(out=tPY[:], in0=tFX[:], scalar1=126.0, scalar2=None, op0=AL.is_le)
            tt(out=tB[:], in0=tB[:], in1=tPY[:], op=AL.mult)
            # bx0 = (1-wx)*vx0 -> tPX ; bx1 = wx*vx1 -> tFRX (in place)
            ts(out=tPX[:], in0=tFRX[:], scalar1=-1.0, scalar2=1.0, op0=AL.mult, op1=AL.add)
            tt(out=tPX[:], in0=tPX[:], in1=tA[:], op=AL.mult)
            tt(out=tFRX[:], in0=tFRX[:], in1=tB[:], op=AL.mult)
            # corner weights [tk, slot]
            w4 = w36[:].rearrange("p (tk s) -> p tk s", s=4)
            tt(out=w4[:, :, 0:1], in0=u3(tC), in1=u3(tPX), op=AL.mult)
            tt(out=w4[:, :, 1:2], in0=u3(tC), in1=u3(tFRX), op=AL.mult)
            tt(out=w4[:, :, 2:3], in0=u3(tD), in1=u3(tPX), op=AL.mult)
            tt(out=w4[:, :, 3:4], in0=u3(tD), in1=u3(tFRX), op=AL.mult)
            # table indices
            ts(out=tA[:], in0=tF[:], scalar1=64.0, scalar2=None, op0=AL.mult)
            tt(out=tA[:], in0=tA[:], in1=tFX[:], op=AL.add)
            ts(out=tA[:], in0=tA[:], scalar1=S_CONST, scalar2=None, op0=AL.add)
            ts(out=tA[:], in0=tA[:], scalar1=IDX_LO, scalar2=IDX_HI, op0=AL.max, op1=AL.min)
            cp(out=idx[:, :], in_=tA[:]).then_inc(sIdx, 1)
            v.drain()

            # main loop (software-pipelined): scalings(t); addB(t-1); drain;
            # addA(t)
            for t in range(NPT + 1):
                b = t % 2
                if t < NPT:
                    v.wait_ge(sGatA, 16 * K * (t // 2 + 1))
                    v.wait_ge(sGatB, 16 * K * ((t + 1) // 2))
                    for ks in range(NDVE_SCL):
                        v.tensor_scalar(
                            out=g2[b][:, ks * C : (ks + 1) * C],
                            in0=g2[b][:, ks * C : (ks + 1) * C],
                            scalar1=w36[:, t * 36 + ks : t * 36 + ks + 1],
                            scalar2=None,
                            op0=AL.mult,
                        )
                v.drain()
                if t >= 1:
                    u = t - 1
                    if u >= 2:
                        # val2[u%2] consumed by the DMA-transpose of tile u-2
                        v.wait_ge(sTpD[u % 2], 16 * (u // 2))
                    val3 = (
                        val2[u % 2][:].rearrange("p (k c) -> p k c", c=C).unsqueeze(2)
                    )
                    va4p = va2[u % 2][:].rearrange("p (k r c) -> p k r c", r=2, c=C)
                    v.tensor_tensor(
                        out=val3,
                        in0=va4p[:, :, 0:1, :],
                        in1=va4p[:, :, 1:2, :],
                        op=AL.add,
                    ).then_inc(sBl, 1)
                if t < NPT:
                    v.wait_ge(sScl, 16 * (t + 1))
                    g4 = g2[b][:].rearrange("p (k s c) -> p k s c", s=4, c=C)
                    va4 = va2[b][:].rearrange("p (k r c) -> p k r c", r=2, c=C)
                    v.tensor_tensor(
                        out=va4, in0=g4[:, :, 0:2, :], in1=g4[:, :, 2:4, :], op=AL.add
                    )

        @block.scalar
        def _(sc):
            for t in range(NPT + 1):
                if t % ST == 0 and t > 0:
                    st = t // ST - 1
                    for oh in range(2):
                        sc.wait_ge(sMM, 2 * st + oh + 1)
                        if st >= 1:
                            sc.wait_ge(sOD, 32 * st)
                        sc.copy(out=ob2[oh][:, :], in_=po2[oh][:, :]).then_inc(sOB, 1)
                if t < NPT:
                    b = t % 2
                    sc.wait_ge(sGatA, 16 * K * (t // 2 + 1))
                    sc.wait_ge(sGatB, 16 * K * ((t + 1) // 2))
                    for ks in range(NDVE_SCL, 36):
                        sc.mul(
                            out=g2[b][:, ks * C : (ks + 1) * C],
                            in_=g2[b][:, ks * C : (ks + 1) * C],
                            mul=w36[:, t * 36 + ks : t * 36 + ks + 1],
                        ).then_inc(sScl, 1)

        @block.tensor
        def _(pe):
            pe.wait_ge(sSet, 64)
            for st in range(NST):
                pe.wait_ge(sTpD0, 16 * (2 * st + 2))
                pe.wait_ge(sTpD1, 16 * (2 * st + 2))
                if st >= 1:
                    pe.wait_ge(sOB, 2 * st)
                for oh in range(2):
                    for j in range(NJ):
                        inst = pe.matmul(
                            out=po2[oh][:, :],
                            lhsT=wts[:, j * 256 + oh * 128 : j * 256 + oh * 128 + 128],
                            rhs=valt2[st % 2][:, j * 512 : (j + 1) * 512],
                            start=(j == 0),
                            stop=(j == NJ - 1),
                        )
                        if j == NJ - 1:
                            inst.then_inc(sMM, 1)

    return nc


def _build_graph():
    if "nc" in _GRAPH_CACHE:
        return _GRAPH_CACHE["nc"]
    nc = bass.Bass("TRN2", debug=False)
    _emit(nc)
    _GRAPH_CACHE["nc"] = nc
    return nc


def _host_prep(x, offset, weight):
    ky = np.repeat(np.array([-1.0, 0.0, 1.0], np.float32), 3)
    kx = np.tile(np.array([-1.0, 0.0, 1.0], np.float32), 3)
    wt = np.ascontiguousarray(
        weight.reshape(O, C, K).transpose(2, 1, 0).reshape(K * C, O)
    ).astype(np.float32)
    in_maps = []
    for core in range(8):
        n, half = core // 2, core % 2
        pos = np.arange(half * P_SH, (half + 1) * P_SH)
        hh = (pos // W).astype(np.float32)
        ww = (pos % W).astype(np.float32)
        basey = hh[:, None] + ky[None, :] + BIAS
        basex = ww[:, None] + kx[None, :] + BIAS
        in_maps.append(
            {
                "xt": np.ascontiguousarray(x[n].reshape(C, S).T).astype(np.float32),
                "wt": wt,
                "offs": np.ascontiguousarray(
                    offset[n, half * P_SH : (half + 1) * P_SH, :]
                ).astype(np.float32),
                "basey": np.ascontiguousarray(basey, np.float32),
                "basex": np.ascontiguousarray(basex, np.float32),
            }
        )
    return in_maps


def kernel(x, offset, weight):
    x = np.asarray(x, np.float32)
    offset = np.asarray(offset, np.float32)
    weight = np.asarray(weight, np.float32)
    nc = _build_graph()
    in_maps = _host_prep(x, offset, weight)
    trace = os.environ.get("BASS_KERNEL_TRACE", "0") == "1"
    try:
        res = run_bass_kernel_spmd(nc, in_maps, core_ids=list(range(8)), trace=trace)
    except ModuleNotFoundError:
        trace = False
        res = run_bass_kernel_spmd(nc, in_maps, core_ids=list(range(8)), trace=False)
    if trace and res.exec_time_ns is not None:
        print(f"HW exec time: {res.exec_time_ns} ns")
        _GRAPH_CACHE["exec_time_ns"] = res.exec_time_ns
        _GRAPH_CACHE["profile"] = res
    outp = np.empty((N, O, H, W), np.float32)
    for core in range(8):
        n, half = core // 2, core % 2
        outp[n].reshape(O, S)[:, half * P_SH : (half + 1) * P_SH] = res.results[core][
            "out"
        ]
    return outp
